# revision 2
# baseline (speedup 1.0000x reference)
"""Trainium2 Bass kernel for the Koopman operator — PWL-refactored.

Each per-channel MLP (and the exp/cos/sin head) is a smooth scalar->scalar
function; host-side we refit each one as a piecewise-linear ReLU expansion
  f(x) = c0 + sum_k a_k * relu(x - t_k)
so the device evaluates ONE hidden layer of 64 shared knot-rows instead of
the original 2x(64x64) hidden layers + transcendentals.

Per-core layout (8192 elements), macro-tile = 2048 elements = 16 groups
of 128:
  x_nat [128, 4 chunks, 4 slots, 32] bf16   (zr | m | 1 | pad per slot)
  4 PE transposes -> xT [128, 4, 128] bf16 (slot s at partition base 32s)
  4 first-matmuls (row+col tiled, K=11, N=512) -> h_ps [128(2x64 knots), 1024]
  relu (ACT/DVE split) -> h_sb bf16
  16 h-stationary final matmuls (K=64, N=32) -> T_ps [128, 16, 32]
    cols 0..15  = P  (lam0..3, A0,A0, .., A5,A5)
    cols 16..31 = Q' (0x4, -B0,B0, .., -B5,B5)
  combine: out = z*P + pairswap(z*Q')
"""

import os
import numpy as np
import ml_dtypes

VARIANT = os.environ.get("KPWL_VARIANT", "")
STAGE = int(os.environ.get("KPWL_STAGE", "6"))
BF16 = ml_dtypes.bfloat16

NR, NCC, L, H = 4, 6, 2, 64
B, S, C = 32, 2048, 16
NCORES = 8
F_CORE = B * S // NCORES          # 8192
MACRO = 2048                      # elements per macro-tile
NT = F_CORE // MACRO              # 4
KR, KC = 8, 5                     # knots per real / complex channel
NKNOT = 4 * KR + 6 * KC           # 62 (+1 const row, +1 pad = 64)
CONST_ROW = 62

R_KNOTS = np.array([-3.6, -2.4, -1.4, -0.6, 0.6, 1.4, 2.4, 3.6])
M_KNOTS = np.array([-0.1, 0.5, 1.4, 3.2, 7.5])

_cached_nc = None


# ----------------------------------------------------------------- host fit
def _mlp_scalar(x, W0, b0, Wm, bm, Wl, bl, p):
    h = np.maximum(x[:, None] * W0[p][None] + b0[p][None], 0)
    for l in range(Wm.shape[0]):
        h = np.maximum(h @ Wm[l, p] + bm[l, p][None], 0)
    return h @ Wl[p] + bl[p][None]


def _fit_pwl(xs, ys, knots, w):
    A = np.concatenate(
        [np.ones((len(xs), 1)), np.maximum(xs[:, None] - knots[None], 0)], axis=1)
    coef, *_ = np.linalg.lstsq(A * w[:, None], ys * w[:, None], rcond=None)
    return coef          # [K+1, out]: row 0 = const


def _pack_weights(i):
    f64 = np.float64
    W0_r, b0_r = f64(i["W0_r"]), f64(i["b0_r"])
    Wm_r, bm_r = f64(i["Wm_r"]), f64(i["bm_r"])
    Wl_r, bl_r = f64(i["Wl_r"]), f64(i["bl_r"])
    W0_c, b0_c = f64(i["W0_c"]), f64(i["b0_c"])
    Wm_c, bm_c = f64(i["Wm_c"]), f64(i["bm_c"])
    Wl_c, bl_c = f64(i["Wl_c"]), f64(i["bl_c"])

    # knots quantized to bf16 first so t is exact on device
    rk = np.asarray(R_KNOTS, BF16).astype(f64)
    mk = np.asarray(M_KNOTS, BF16).astype(f64)

    # knot index map: k in [0,32) real ch k//8; [32,62) m ch 4+(k-32)//5
    tvec = np.zeros(64)
    chan = np.zeros(64, np.int32)
    for k in range(32):
        tvec[k] = rk[k % KR]
        chan[k] = k // KR
    for k in range(32, 62):
        tvec[k] = mk[(k - 32) % KC]
        chan[k] = 4 + (k - 32) // KC

    # per-channel PWL coefficients
    gx = np.linspace(-5.2, 5.2, 2001)
    gw = (np.abs(gx) + 0.3) * np.exp(-gx ** 2 / 4) + 0.01
    lam_coef = []
    for p in range(4):
        gy = _mlp_scalar(gx, W0_r, b0_r, Wm_r, bm_r, Wl_r, bl_r, p)[:, 0:1]
        lam_coef.append(_fit_pwl(gx, gy, rk, gw))
    gxm = np.linspace(0.0, 28.0, 2001)
    gwm = (np.sqrt(gxm) + 0.4) * np.exp(-gxm / 4) + 0.01
    AB_coef = []
    for p in range(6):
        mo = _mlp_scalar(gxm, W0_c, b0_c, Wm_c, bm_c, Wl_c, bl_c, p)
        Af = np.exp(mo[:, 0]) * np.cos(mo[:, 1])
        Bf = np.exp(mo[:, 0]) * np.sin(mo[:, 1])
        cA = _fit_pwl(gxm, Af[:, None], mk, gwm)
        cB = _fit_pwl(gxm, Bf[:, None], mk, gwm)
        AB_coef.append((cA, cB))

    # ---- w0rep [128, 64]: 4 replicas at partition bases 0/32/64/96.
    # rows base+0..9 channel selectors; bias (-t) rides the relu bias AP.
    w0rep = np.zeros((128, 64), f64)
    for base in (0, 32, 64, 96):
        for k in range(62):
            w0rep[base + chan[k], k] = 1.0

    # ---- afull [128, 32]: two vertical replicas (knot-parity halves).
    # cols 0..15 = P, cols 16..31 = Q'.
    a64 = np.zeros((64, 32), f64)
    for k in range(62):
        ch = chan[k]
        if ch < 4:                      # real: P col = ch
            a64[k, ch] = lam_coef[ch][1 + (k % KR), 0]
        else:                           # complex pair i
            ip = ch - 4
            cA, cB = AB_coef[ip]
            ak = 1 + (k - 32) % KC
            a64[k, 4 + 2 * ip] = cA[ak, 0]
            a64[k, 5 + 2 * ip] = cA[ak, 0]
            a64[k, 16 + 4 + 2 * ip] = -cB[ak, 0]   # multiplies z1 -> -B
            a64[k, 16 + 5 + 2 * ip] = cB[ak, 0]    # multiplies z2 -> +B
    # const row: function constants
    for ch in range(4):
        a64[CONST_ROW, ch] = lam_coef[ch][0, 0]
    for ip in range(6):
        cA, cB = AB_coef[ip]
        a64[CONST_ROW, 4 + 2 * ip] = cA[0, 0]
        a64[CONST_ROW, 5 + 2 * ip] = cA[0, 0]
        a64[CONST_ROW, 16 + 4 + 2 * ip] = -cB[0, 0]
        a64[CONST_ROW, 16 + 5 + 2 * ip] = cB[0, 0]
    # parity-masked: cols 0:32 use knot rows 0..63, cols 32:64 rows 64..127
    afp = np.zeros((128, 64), f64)
    afp[0:64, 0:32] = a64
    afp[64:128, 32:64] = a64

    # relu bias: -t per knot row (both parity halves); const knot row gets +1
    tv = np.zeros((128, 1), np.float32)
    tv[0:62, 0] = -tvec[:62]
    tv[CONST_ROW, 0] = 1.0
    tv[64:128] = tv[0:64]
    ident = np.zeros((128, 128), BF16)
    np.fill_diagonal(ident, 1.0)
    return {"w0rep": np.asarray(w0rep, BF16), "afull": np.asarray(afp, BF16),
            "tvec": tv, "identity": ident}


# ----------------------------------------------------------------- device
def _build():
    import concourse.tile as tile
    from concourse import bacc, mybir
    from concourse.masks import make_identity

    f32 = mybir.dt.float32
    bf16 = mybir.dt.bfloat16
    RELU = mybir.ActivationFunctionType.Relu
    MAX = mybir.AluOpType.max

    nc = bacc.Bacc("TRN2", target_bir_lowering=False, debug=False,
                   num_devices=NCORES)

    z = nc.dram_tensor("z", [NT, 128, 4, 4, C], f32, kind="ExternalInput").ap()
    w0rep_d = nc.dram_tensor("w0rep", [128, 64], bf16, kind="ExternalInput").ap()
    afull_d = nc.dram_tensor("afull", [128, 64], bf16, kind="ExternalInput").ap()
    tvec_d = nc.dram_tensor("tvec", [128, 1], f32, kind="ExternalInput").ap()
    ident_d = nc.dram_tensor("identity", [128, 128], bf16,
                             kind="ExternalInput").ap()
    out = nc.dram_tensor("out", [NT, 128, 4, 4, C], f32, kind="ExternalOutput").ap()

    z_r = z
    out_r = out

    RELU_SPLIT = 768   # cols 0..SPLIT on ACT, rest on DVE

    with tile.TileContext(nc) as tc:
        with (
            tc.tile_pool(name="singles", bufs=1) as singles,
            tc.tile_pool(name="io", bufs=3) as io,
            tc.tile_pool(name="xb", bufs=3) as xb,
            tc.tile_pool(name="hsb", bufs=2) as hsb,
            tc.tile_pool(name="psA", bufs=2, space="PSUM") as psA,
            tc.tile_pool(name="psH", bufs=2, space="PSUM") as psH,
            tc.tile_pool(name="psT", bufs=2, space="PSUM") as psT,
        ):
            z_tiles = []
            for t in range(NT):
                zt = singles.tile([128, 4, 4, 16], f32, tag=f"z{t}")
                z_tiles.append(zt)
            # macro-0 halves first on both queues, then weights, then rest
            nc.sync.dma_start(out=z_tiles[0][:, 0:2], in_=z_r[0][:, 0:2])
            nc.scalar.dma_start(out=z_tiles[0][:, 2:4], in_=z_r[0][:, 2:4])
            ident = singles.tile([128, 128], bf16, tag="ident")
            nc.sync.dma_start(out=ident, in_=ident_d)
            w0rep = singles.tile([128, 64], bf16, tag="w0rep")
            nc.scalar.dma_start(out=w0rep, in_=w0rep_d)
            afull = singles.tile([128, 64], bf16, tag="afull")
            nc.sync.dma_start(out=afull, in_=afull_d)
            tvec = singles.tile([128, 1], f32, tag="tvec")
            nc.scalar.dma_start(out=tvec, in_=tvec_d)
            for t in range(1, NT):
                eng = nc.sync if t % 2 == 1 else nc.scalar
                eng.dma_start(out=z_tiles[t], in_=z_r[t])

            for t in range(NT):
                z_nat = z_tiles[t]

                if STAGE == 1:
                    o_nat = io.tile([128, 4, 4, 16], f32, tag="o")
                    nc.vector.tensor_copy(o_nat, z_nat)
                    for q in range(4):
                        nc.sync.dma_start(out=out_r[t][:, q], in_=o_nat[:, q])
                    continue

                # x_nat: per (chunk q, slot s): [zr(4) | m(6) | 1 | pad]
                x_nat = xb.tile([128, 4, 4, 32], bf16, tag="x")
                sq = xb.tile([128, 4, 4, 12], f32, tag="sq")
                nc.gpsimd.tensor_mul(sq, z_nat[:, :, :, 4:16],
                                     z_nat[:, :, :, 4:16])
                nc.gpsimd.tensor_add(x_nat[:, :, :, 4:10],
                                     sq[:, :, :, 0:12:2], sq[:, :, :, 1:12:2])
                nc.gpsimd.tensor_copy(x_nat[:, :, :, 0:4], z_nat[:, :, :, 0:4])

                if STAGE == 2:
                    o_nat = io.tile([128, 4, 4, 16], f32, tag="o")
                    nc.vector.tensor_copy(o_nat, x_nat[:, :, :, 0:16])
                    for q in range(4):
                        nc.sync.dma_start(out=out_r[t][:, q], in_=o_nat[:, q])
                    continue

                # transpose each chunk: [128e, 128(s,c)] -> [128(s,c), 128e]
                xT_ps = psA.tile([128, 4, 128], bf16, tag="xT")
                for q in range(4):
                    nc.tensor.transpose(xT_ps[:, q], x_nat[:, q], ident)
                xT_sb = xb.tile([128, 4, 128], bf16, tag="xTs")
                nc.vector.tensor_copy(xT_sb, xT_ps)

                if STAGE == 3:
                    o_nat = io.tile([128, 4, 4, 16], f32, tag="o")
                    nc.vector.tensor_copy(
                        o_nat, xT_sb.rearrange("p a b -> p (a b)")[:, 0:256]
                        .rearrange("p (a b c) -> p a b c", a=4, b=4))
                    for q in range(4):
                        nc.sync.dma_start(out=out_r[t][:, q], in_=o_nat[:, q])
                    continue

                # first matmuls: slot s -> knot-parity s%2, col-half s//2
                h_psA = psH.tile([128, 512], f32, tag="hA")
                h_psB = psH.tile([128, 512], f32, tag="hB")
                h_pss = [h_psA, h_psB]
                for s in range(4):
                    pi, j = s % 2, s // 2
                    nc.tensor.matmul(
                        h_pss[j][64 * pi:64 * pi + 64, :],
                        lhsT=w0rep[32 * s:32 * s + 10, :],
                        rhs=xT_sb[32 * s:32 * s + 10, :, :],
                        start=True, stop=True,
                        tile_position=(32 * s, 64 * pi))

                h_sbA = hsb.tile([128, 512], bf16, tag="hsA")
                h_sbB = hsb.tile([128, 512], bf16, tag="hsB")
                h_sbs = [h_sbA, h_sbB]
                nc.scalar.activation(h_sbs[0], h_pss[0], RELU, bias=tvec)
                nc.scalar.activation(h_sbs[1], h_pss[1], RELU, bias=tvec)

                if STAGE == 4:
                    o_nat = io.tile([128, 4, 4, 16], f32, tag="o")
                    nc.vector.tensor_copy(
                        o_nat, h_sb[:, 0:256]
                        .rearrange("p (a b c) -> p a b c", a=4, b=4))
                    for q in range(4):
                        nc.sync.dma_start(out=out_r[t][:, q], in_=o_nat[:, q])
                    continue

                # final matmuls: h block for group (q, s) at cols
                # 512*(s//2) + 128*q, knot rows at base 64*(s%2)
                T_ps = psT.tile([128, 4, 4, 32], f32, tag="T")
                for g in range(16):
                    q, s = g // 4, g % 4
                    pi = s % 2
                    nc.tensor.matmul(
                        T_ps[:, q, s, :],
                        lhsT=h_sbs[s // 2][:, 128 * q:128 * q + 128],
                        rhs=afull[:, 32 * pi:32 * pi + 32],
                        start=True, stop=True)

                if STAGE == 5:
                    o_nat = io.tile([128, 4, 4, 16], f32, tag="o")
                    nc.vector.tensor_copy(o_nat, T_ps[:, :, :, 0:16])
                    for q in range(4):
                        nc.sync.dma_start(out=out_r[t][:, q], in_=o_nat[:, q])
                    continue

                # combine: out = z*P + pairswap(z*Q')
                t1 = io.tile([128, 4, 4, 16], f32, tag="t1")
                t2 = io.tile([128, 4, 4, 16], f32, tag="t2")
                o_nat = io.tile([128, 4, 4, 16], f32, tag="o")
                nc.vector.tensor_mul(t1, z_nat, T_ps[:, :, :, 0:16])
                nc.vector.tensor_mul(t2, z_nat, T_ps[:, :, :, 16:32])
                if VARIANT == "noswap":
                    nc.vector.tensor_add(o_nat, t1, t2)
                else:
                    t2v = t2.rearrange("p a b (r w) -> p a b r w", w=2)
                    t2s = t2v[:, :, :, :, ::-1]
                    t1v = t1.rearrange("p a b (r w) -> p a b r w", w=2)
                    ov = o_nat.rearrange("p a b (r w) -> p a b r w", w=2)
                    nc.vector.tensor_add(ov, t1v, t2s)

                if t < NT - 1:
                    nc.sync.dma_start(out=out_r[t], in_=o_nat)
                else:
                    for q in range(4):
                        eng = nc.sync if q % 2 == 0 else nc.scalar
                        eng.dma_start(out=out_r[t][:, q], in_=o_nat[:, q])

    nc.compile()
    return nc


def _prepare_in_maps(inputs):
    weights = _pack_weights(inputs)
    # device layout: [NT, p, q, s, c] with p the 128-partition dim
    z = (np.asarray(inputs["z"], np.float32)
         .reshape(NCORES, NT, 4, 4, 128, C)     # (core, t, q, s, p, c)
         .transpose(0, 1, 4, 2, 3, 5))          # -> (core, t, p, q, s, c)
    z = np.ascontiguousarray(z)
    return [dict(weights, z=z[i]) for i in range(NCORES)]


def _unshard_out(results):
    outs = [np.asarray(results[i]["out"]) for i in range(NCORES)]
    o = np.stack(outs, axis=0)                  # (core, t, p, q, s, c)
    o = o.transpose(0, 1, 3, 4, 2, 5)           # -> (core, t, q, s, p, c)
    return np.ascontiguousarray(o).reshape(B, S, C)


def kernel(**inputs):
    global _cached_nc
    if _cached_nc is None:
        _cached_nc = _build()
    nc = _cached_nc

    from concourse.bass_utils import run_bass_kernel_spmd

    in_maps = _prepare_in_maps(inputs)
    res = run_bass_kernel_spmd(nc, in_maps, core_ids=list(range(NCORES)))
    return _unshard_out(res.results)
